# revision 49
# baseline (speedup 1.0000x reference)
"""Trainium2 Bass kernel for nn_Attention_10015863734775.

Multi-head causal attention (16 heads, d_model 2048, d_head 128, seq 2048,
batch 1) with llama-style interleaved RoPE and a signed-softmax:
    attn_w = sign(s) * exp(|s| - max|s|);  attn = attn_w / (sum|attn_w| + 1e-6)
The max-subtraction cancels in the normalization, so the device computes
attn = sign(s)exp(|s|) / sum exp(|s|).

Sharding: 2 heads per NeuronCore (8 cores). Each core receives the full
transposed residual X^T plus its head slices of W_Q/K/V/W_O and computes a
partial output projection outT_c[m, s]; the host sums the 8 partials,
transposes, and adds b_O.

This version keeps the PE saturated end-to-end (the HAM throttle un-gates
only under continuous PE activity): the QKV-projection (A), attention (B)
and output-projection (C) phases are emitted as one interleaved stream.
A-phase runs one head at a time in two passes, q/k then v (2 PSUM
accumulator banks), so B's z/denom
accumulators can coexist; PSUM budget is exactly 8 banks:
  a0 a1 | s s (scores/rope-shuffle/v-transpose/psO ring) | z0 z1 | d0 d1
Each head's denominator accumulates broadcast across a full bank (ones
stationary), so its fast-approx reciprocal is directly the broadcast
normalizer.
Elementwise work is greedily balanced across ACT / DVE / GpSimd.
"""

import math

import numpy as np

S = 2048          # sequence length
D = 2048          # d_model
DH = 128          # d_head
NH = 16           # total heads
NC = 8            # neuron cores
HPC = NH // NC    # heads per core (2)
ST = 512          # seq tile (matmul free dim / one PSUM bank)
NST = S // ST     # 4 seq tiles
NDC = D // 128    # 16 contraction chunks
C_SCALE = 1.0 / math.sqrt(float(DH))
MASK_NEG = -1.0e5

_CACHE = {}


def _build_program():
    import concourse.tile as tile
    from concourse import bacc, mybir

    F32 = mybir.dt.float32
    F32R = mybir.dt.float32r
    U32 = mybir.dt.uint32
    AF = mybir.ActivationFunctionType
    ALU = mybir.AluOpType

    nc = bacc.Bacc("TRN2", target_bir_lowering=False, debug=False, num_devices=NC)

    xt_d = nc.dram_tensor("xt", [D, S], F32, kind="ExternalInput").ap()
    wall_d = nc.dram_tensor("wall", [128, NDC * 6 * DH], F32, kind="ExternalInput").ap()
    wo_d = nc.dram_tensor("wo", [HPC, DH, D], F32, kind="ExternalInput").ap()
    bq_d = nc.dram_tensor("bq", [HPC, DH, 1], F32, kind="ExternalInput").ap()
    bk_d = nc.dram_tensor("bk", [HPC, DH, 1], F32, kind="ExternalInput").ap()
    bv_d = nc.dram_tensor("bv", [HPC, DH, 1], F32, kind="ExternalInput").ap()
    cos_d = nc.dram_tensor("cost", [DH, S], F32, kind="ExternalInput").ap()
    sin_d = nc.dram_tensor("sint", [DH, S], F32, kind="ExternalInput").ap()
    tri_d = nc.dram_tensor("tri", [128, 128], F32, kind="ExternalInput").ap()
    pt_d = nc.dram_tensor("pt", [128, 128], F32, kind="ExternalInput").ap()
    id_d = nc.dram_tensor("ident", [128, 128], F32, kind="ExternalInput").ap()
    ok_d = nc.dram_tensor("onesk", [128, 128], F32, kind="ExternalInput").ap()
    BF16 = mybir.dt.bfloat16
    out_d = nc.dram_tensor("outt", [D, S], BF16, kind="ExternalOutput").ap()

    # engine load model (us per full-width [128,512] pass) for greedy balance
    load = {"act": 0.0, "dve": 0.0, "gp": 0.0}

    def pick(*opts):
        e, c = min(opts, key=lambda ec: load[ec[0]] + ec[1])
        load[e] += c
        return e

    with tile.TileContext(nc) as tc:
        with tc.tile_pool(name="persist", bufs=1) as pp, \
             tc.tile_pool(name="sp", bufs=1) as sp, \
             tc.tile_pool(name="psA", bufs=1, space="PSUM") as psA, \
             tc.tile_pool(name="psS", bufs=2, space="PSUM") as psS, \
             tc.tile_pool(name="psZ", bufs=1, space="PSUM") as psZ, \
             tc.tile_pool(name="psD", bufs=1, space="PSUM") as psD:

            # ---------------- persistent SBUF -----------------------------
            wall_sb = pp.tile([128, NDC * 6 * DH], F32R, tag="wall", name="wall_sb")
            krot = [pp.tile([DH, S], F32R, tag=f"krot{h}", name=f"krot{h}")
                    for h in range(HPC)]
            v_sb = [pp.tile([128, NDC * DH], F32R, tag=f"v{h}", name=f"v{h}")
                    for h in range(HPC)]
            wo_sb = [pp.tile([DH, D], F32R, tag=f"wo{h}", name=f"wo{h}")
                     for h in range(HPC)]
            bqs, bks, bvs = [], [], []
            for h in range(HPC):
                for lst, dd, nm in ((bqs, bq_d, "bq"), (bks, bk_d, "bk"),
                                    (bvs, bv_d, "bv")):
                    bt = pp.tile([DH, 1], F32, tag=f"{nm}{h}", name=f"{nm}{h}")
                    lst.append(bt)
            tri_sb = pp.tile([128, 128], F32, tag="tri", name="tri_sb")
            pt_sb = pp.tile([128, 128], F32R, tag="pt", name="pt_sb")
            id_sb = pp.tile([128, 128], F32R, tag="ident", name="id_sb")
            ok_sb = pp.tile([128, 128], F32R, tag="onesk", name="ok_sb")
            cabs_sb = pp.tile([128, 1], U32, tag="cabs", name="cabs_sb")
            nc.vector.memset(cabs_sb[:], 0x7FFFFFFF)
            csgn_sb = pp.tile([128, 1], U32, tag="csgn", name="csgn_sb")
            nc.vector.memset(csgn_sb[:], 0x80000000)
            cone_sb = pp.tile([128, 1], U32, tag="cone", name="cone_sb")
            nc.vector.memset(cone_sb[:], 0x3F800000)
            zero_sb = pp.tile([128, 1], F32, tag="zero", name="zero_sb")
            nc.vector.memset(zero_sb[:], 0.0)
            import os as _os
            _nonce = float(int(_os.environ.get("KBUILD_NONCE", "0")))
            nonce_sb = pp.tile([128, 1], F32, tag="nonce", name="nonce_sb")
            nc.vector.memset(nonce_sb[:], _nonce)

            CH = 768  # f32 cols per dc chunk (6*128)

            def dma_wall(dc, piece):
                # host wall layout per dc: [q0 k0 | v0 v1 | q1 k1]; three
                # 256-col sweeps on the scalar hwdge queue, ordered to match
                # the A(0) pass consumption (h0-qk, v, h1-qk)
                o = dc * CH + piece * 256
                nc.scalar.dma_start(wall_sb[:, o:o + 256],
                                    wall_d[:, o:o + 256].bitcast(F32R))

            _wix = ((0, 1, 2), (4, 5, 3))  # [h][q,k,v] -> stream slot

            def wsl(dc, h, key):
                o = (dc * 6 + _wix[h][key]) * 128
                return wall_sb[:, o:o + 128]

            # ---------------- working pools --------------------------------
            # xt tiles for seq-tile st stay live across both head sub-bursts
            xt_p = sp  # tags below carry bufs explicitly
            qrot_p = sp
            trig_p = sp
            work_p = sp
            out_p = sp

            xt_tiles = {}     # (st, dc) -> tile (ring bufs=17)
            qrot_t = {}       # (st, h) -> tile
            znt_t = {}        # (j, h) -> tile
            cos_t = {}        # st -> tile
            sin_t = {}

            def dma_xt(st, dc):
                t = xt_p.tile([128, ST], F32R, tag="xt", bufs=17,
                              name=f"xt{st}_{dc}")
                nc.sync.dma_start(
                    t[:], xt_d[dc * 128:(dc + 1) * 128,
                               st * ST:(st + 1) * ST].bitcast(F32R))
                xt_tiles[(st, dc)] = t

            def dma_trig(st):
                ssl = slice(st * ST, (st + 1) * ST)
                ct = trig_p.tile([DH, ST], F32, tag="cos", bufs=2, name=f"cos{st}")
                nc.scalar.dma_start(ct[:], cos_d[:, ssl])
                st_t = trig_p.tile([DH, ST], F32, tag="sin", bufs=2, name=f"sin{st}")
                nc.scalar.dma_start(st_t[:], sin_d[:, ssl])
                cos_t[st] = ct
                sin_t[st] = st_t

            # ---------------- A phase work units ---------------------------
            acc = {}

            def a_mm_qk(st, h, dc):
                if dc == 0:
                    acc["q"] = psA.tile([128, ST], F32, tag="a0", name=f"aq{st}_{h}")
                    acc["k"] = psA.tile([128, ST], F32, tag="a1", name=f"ak{st}_{h}")
                xt_t = xt_tiles[(st, dc)]
                for ki, key in enumerate(("q", "k")):
                    nc.tensor.matmul(acc[key][:], wsl(dc, h, ki), xt_t[:],
                                     start=(dc == 0), stop=(dc == NDC - 1))

            def a_mm_v(st, h, dc):
                if dc == 0:
                    acc["v"] = psA.tile([128, ST], F32, tag="a0", name=f"av{st}_{h}")
                xt_t = xt_tiles[(st, dc)]
                nc.tensor.matmul(acc["v"][:], wsl(dc, h, 2), xt_t[:],
                                 start=(dc == 0), stop=(dc == NDC - 1))

            def a_tail_qk(st, h):
                """q/k evictions + rope for (st, h)"""
                ssl = slice(st * ST, (st + 1) * ST)
                for key, bias in (("q", bqs[h]), ("k", bks[h])):
                    x_sb = work_p.tile([128, ST], F32R, tag="ev", bufs=3,
                                       name=f"ev{key}{h}_{st}")
                    if st == 0 or pick(("dve", 0.78), ("act", 0.70)) == "dve":
                        if st == 0:
                            load["dve"] += 0.78
                        nc.vector.tensor_scalar(x_sb[:], acc[key][:], bias[:],
                                                None, ALU.add)
                    else:
                        nc.scalar.activation(x_sb[:], acc[key][:], AF.Identity,
                                             bias=bias[:])
                    if key == "q":
                        dst = qrot_p.tile([DH, ST], F32R, tag=f"qr{h}", bufs=2,
                                          name=f"qr{h}_{st}")
                        qrot_t[(st, h)] = dst
                        dsl = slice(0, ST)
                    else:
                        dst = krot[h]
                        dsl = ssl
                    shuf = psS.tile([128, ST], F32, tag="s", name=f"sh{key}{h}_{st}")
                    nc.tensor.matmul(shuf[:], pt_sb[:], x_sb[:], start=True,
                                     stop=True)
                    t1 = work_p.tile([128, ST], F32, tag="t1", bufs=2,
                                     name=f"t1{key}{h}_{st}")
                    load["gp"] += 1.45
                    nc.gpsimd.tensor_tensor(t1[:], x_sb[:].bitcast(F32),
                                            cos_t[st][:], ALU.mult)
                    t2 = work_p.tile([128, ST], F32, tag="t2", bufs=2,
                                     name=f"t2{key}{h}_{st}")
                    load["dve"] += 0.90
                    nc.vector.tensor_tensor(t2[:], shuf[:], sin_t[st][:], ALU.mult)
                    if pick(("dve", 0.90), ("gp", 1.45)) == "dve":
                        nc.vector.tensor_tensor(dst[:, dsl], t1[:], t2[:], ALU.add)
                    else:
                        nc.gpsimd.tensor_tensor(dst[:, dsl], t1[:], t2[:], ALU.add)

            def a_tail_v(st, h):
                """v eviction + transpose for (st, h)"""
                vt_sb = work_p.tile([128, ST], F32R, tag="evv", bufs=2,
                                    name=f"evv{h}_{st}")
                load["dve"] += 0.78
                nc.vector.tensor_scalar(vt_sb[:], acc["v"][:], bvs[h][:], None,
                                        ALU.add)
                vtr = psS.tile([128, ST], F32, tag="s", name=f"vtr{h}_{st}")
                for sc in range(4):
                    nc.tensor.transpose(
                        vtr[:, sc * 128:(sc + 1) * 128].bitcast(F32R),
                        vt_sb[:, sc * 128:(sc + 1) * 128], id_sb[:])
                if pick(("dve", 0.78), ("act", 0.72)) == "dve":
                    nc.vector.tensor_copy(
                        v_sb[h][:, st * ST:(st + 1) * ST], vtr[:])
                else:
                    nc.scalar.activation(
                        v_sb[h][:, st * ST:(st + 1) * ST], vtr[:], AF.Copy)

            # ---------------- B phase work units ---------------------------
            pss_map = {}
            psz = {}
            psd = {}

            def emit_scores(j, kc, h):
                jj = kc - 4 * j
                lo = jj * 128 if 0 <= jj < 4 else 0
                pss = psS.tile([128, ST], F32, tag="s", name=f"s{h}_{j}_{kc}")
                nc.tensor.matmul(pss[:, lo:], krot[h][:, kc * 128:(kc + 1) * 128],
                                 qrot_t[(j, h)][:, lo:], start=True, stop=True)
                pss_map[(j, kc, h)] = (pss, lo)

            def emit_rest(j, kc, h):
                pss, lo = pss_map.pop((j, kc, h))
                if kc == 0:
                    psz[(j, h)] = psZ.tile([128, ST], F32, tag=f"z{h}",
                                           name=f"z{h}_{j}")
                    psd[(j, h)] = psD.tile([128, ST], F32, tag=f"d{h}",
                                           name=f"d{h}_{j}")
                lsl = slice(lo, ST)
                wdt = ST - lo
                f = wdt / ST
                a = work_p.tile([128, ST], F32, tag="a", bufs=2,
                                name=f"a{h}_{j}_{kc}")
                if pick(("dve", 0.78 * f), ("act", 0.70 * f)) == "dve":
                    nc.vector.tensor_scalar(a[:, lsl].bitcast(U32),
                                            pss[:, lsl].bitcast(U32),
                                            cabs_sb[:], None, ALU.bitwise_and)
                    exp_scale = C_SCALE
                else:
                    nc.scalar.activation(a[:, lsl], pss[:, lsl], AF.Abs,
                                         bias=zero_sb[:], scale=C_SCALE)
                    exp_scale = 1.0
                jj = kc - 4 * j
                if 0 <= jj < 4:
                    load["gp"] += 0.42
                    nc.gpsimd.tensor_tensor(a[:, lo:lo + 128], a[:, lo:lo + 128],
                                            tri_sb[:], ALU.add)
                # sign: on the engine opposite the abs
                sg = work_p.tile([128, ST], F32, tag="sg", bufs=2,
                                 name=f"sg{h}_{j}_{kc}")
                if pick(("dve", 0.78 * f), ("act", 0.70 * f)) == "dve":
                    nc.vector.tensor_scalar(sg[:, lsl].bitcast(U32),
                                            pss[:, lsl].bitcast(U32),
                                            csgn_sb[:], cone_sb[:],
                                            ALU.bitwise_and, ALU.bitwise_or)
                else:
                    nc.scalar.activation(sg[:, lsl], pss[:, lsl], AF.Sign,
                                         bias=zero_sb[:])
                e2 = work_p.tile([128, ST], F32R, tag="e2", bufs=4,
                                 name=f"e2{h}_{j}_{kc}")
                load["act"] += 0.70 * f
                nc.scalar.activation(e2[:, lsl], a[:, lsl], AF.Exp,
                                     bias=zero_sb[:], scale=exp_scale)
                w = work_p.tile([128, ST], F32R, tag="w", bufs=4,
                                name=f"w{h}_{j}_{kc}")
                if pick(("dve", 0.90 * f), ("gp", 1.45 * f)) == "dve":
                    nc.vector.tensor_tensor(w[:, lsl], sg[:, lsl],
                                            e2[:, lsl].bitcast(F32), ALU.mult)
                else:
                    nc.gpsimd.tensor_tensor(w[:, lsl], sg[:, lsl],
                                            e2[:, lsl].bitcast(F32), ALU.mult)
                nkc_ = 4 * (j + 1)
                nc.tensor.matmul(psd[(j, h)][:, lsl], ok_sb[:],
                                 e2[:, lsl], start=(kc == 0), stop=(kc == nkc_ - 1))
                nc.tensor.matmul(psz[(j, h)][:, lsl],
                                 v_sb[h][:, kc * 128:(kc + 1) * 128], w[:, lsl],
                                 start=(kc == 0), stop=(kc == nkc_ - 1))

            def finalize(j, h):
                rb_sb = work_p.tile([128, ST], F32, tag="rb", bufs=2,
                                    name=f"rbs{h}_{j}")
                load["dve"] += 0.62
                nc.vector.reciprocal_approx_fast(out=rb_sb[:],
                                                 in_=psd.pop((j, h))[:])
                zt = qrot_p.tile([DH, ST], F32R, tag=f"znt{h}", bufs=2,
                                 name=f"znt{h}_{j}")
                znt_t[(j, h)] = zt
                load["dve"] += 0.90
                nc.vector.tensor_tensor(zt[:], psz.pop((j, h))[:], rb_sb[:],
                                        ALU.mult)

            def emit_C(j, mc0):
                jsl = slice(j * ST, (j + 1) * ST)
                for mc in (mc0, mc0 + 1):
                    pso = psS.tile([128, ST], F32, tag="s", name=f"o{j}_{mc}")
                    for h in range(HPC):
                        nc.tensor.matmul(pso[:],
                                         wo_sb[h][:, mc * 128:(mc + 1) * 128],
                                         znt_t[(j, h)][:], start=(h == 0),
                                         stop=(h == HPC - 1))
                    o_sb = out_p.tile([128, ST], BF16, tag="o", bufs=2,
                                      name=f"ev{j}_{mc}")
                    if pick(("dve", 0.55), ("act", 0.55)) == "dve":
                        nc.vector.tensor_copy(o_sb[:], pso[:])
                    else:
                        nc.scalar.activation(o_sb[:], pso[:], AF.Copy)
                    nc.sync.dma_start(out_d[mc * 128:(mc + 1) * 128, jsl], o_sb[:])

            # ---------------- global schedule ------------------------------
            # PE warm-up: the HAM throttle needs ~3.4us of continuous PE
            # activity to lift to full rate, and the first real matmul can't
            # start until wall(0)+xt(0,0) land (~9us). Spin the PE on a dummy
            # tile so the p-state/HAM are warm when real work arrives.
            dumf = work_p.tile([128, 256], F32, tag="dumf", bufs=1, name="dumf")
            nc.vector.memset(dumf[:], 1.0)
            dum = work_p.tile([128, 256], F32R, tag="dum", bufs=1, name="dum")
            nc.vector.tensor_copy(dum[:], dumf[:])
            for i in range(30):
                pswu = psS.tile([128, ST], F32, tag="s", name=f"wu{i}")
                nc.tensor.matmul(pswu[:, :128], dum[:, :128], dum[:, :128],
                                 start=True, stop=True)

            # prologue DMAs. scalar hwdge: wall chunks paced to the A(0)
            # consumption, with the early-needed consts slotted between.
            # sync hwdge: xt st=0 then biases/tri/ok. swdge: wo only.
            for dc in range(NDC):
                dma_xt(0, dc)
            for lst, dd in ((bqs, bq_d), (bks, bk_d), (bvs, bv_d)):
                for h in range(HPC):
                    nc.sync.dma_start(lst[h][:], dd[h])
            nc.sync.dma_start(tri_sb[:], tri_d[:])
            nc.sync.dma_start(ok_sb[:], ok_d[:].bitcast(F32R))
            for dc in range(6):
                dma_wall(dc, 0)
            nc.scalar.dma_start(pt_sb[:], pt_d[:].bitcast(F32R))
            nc.scalar.dma_start(id_sb[:], id_d[:].bitcast(F32R))
            for dc in range(6, NDC):
                dma_wall(dc, 0)
            dma_trig(0)
            for dc in range(NDC):
                dma_wall(dc, 1)
            for dc in range(NDC):
                dma_wall(dc, 2)
            for h in range(HPC):
                nc.gpsimd.dma_start(wo_sb[h][:], wo_d[h].bitcast(F32R))

            # A(0, h0): wall-DMA-paced; fill the chase gaps with dummy MMs so
            # the PE/HAM stay warm while weight chunks stream in.
            for dc in range(NDC):
                a_mm_qk(0, 0, dc)
                if dc < 14:
                    pswu = psS.tile([128, ST], F32, tag="s", name=f"wf{dc}")
                    nc.tensor.matmul(pswu[:, :256], dum[:, :128], dum[:, :256],
                                     start=True, stop=True)
            a_tail_qk(0, 0)
            for dc in range(NDC):
                a_mm_v(0, 0, dc)
            a_tail_v(0, 0)

            # ---- globally-paced stream: B blocks paced against one filler
            # stream (A(0,h1), A(1..3), DMAs) plus a C queue gated on fin(j).
            # B(j) blocks require A(<=j) fully emitted (qrot/krot/v deps).
            filler = []
            # A(0, h1)
            for dc in range(NDC):
                filler.append(("aqk", 0, 1, dc, 0.43))
            filler.append(("atailqk", 0, 1, 0.45))
            for dc in range(NDC):
                filler.append(("av", 0, 1, dc, 0.21))
            filler.append(("atailv", 0, 1, 0.35))
            filler.append(("trig", 1, 0.0))
            # A(1..3)
            for stn in range(1, NST):
                for h in range(HPC):
                    for dc in range(NDC):
                        if h == 0:
                            filler.append(("xt", stn, dc, 0.0))
                        filler.append(("aqk", stn, h, dc, 0.43))
                    filler.append(("atailqk", stn, h, 0.45))
                    for dc in range(NDC):
                        filler.append(("av", stn, h, dc, 0.21))
                    filler.append(("atailv", stn, h, 0.35))
                if stn + 1 < NST:
                    filler.append(("trig", stn + 1, 0.0))
            # index of the last filler unit belonging to A(j) (for dep gating)
            a_done_idx = {}
            for i, u in enumerate(filler):
                if u[0] == "atailv":
                    a_done_idx[u[1]] = i
            total_filler_pe = sum(u[-1] for u in filler)

            def run_filler(u):
                kind = u[0]
                if kind == "xt":
                    dma_xt(u[1], u[2])
                elif kind == "aqk":
                    a_mm_qk(u[1], u[2], u[3])
                elif kind == "av":
                    a_mm_v(u[1], u[2], u[3])
                elif kind == "atailqk":
                    a_tail_qk(u[1], u[2])
                elif kind == "atailv":
                    a_tail_v(u[1], u[2])
                elif kind == "trig":
                    dma_trig(u[1])

            blocks = [(j, kc, h) for j in range(NST)
                      for kc in range(4 * (j + 1)) for h in range(HPC)]
            # block weight = elementwise width fraction
            def bw_of(j, kc):
                jj = kc - 4 * j
                return 1.0 - jj * 0.25 if 0 <= jj < 4 else 1.0
            total_bw = sum(bw_of(j, kc) for j, kc, h in blocks)

            C_queue = []     # (j, mc0) ready to emit
            c_emitted = 0
            total_c_pairs = NST * (NDC // 2)
            done_in_j = {j: 0 for j in range(NST)}
            LOOK = 3
            fpos = 0
            have_filler_pe = 0.0
            bw_done = 0.0

            def after_rest(jd):
                done_in_j[jd] += 1
                if done_in_j[jd] == 2 * 4 * (jd + 1):
                    for h in range(HPC):
                        finalize(jd, h)
                    for mc0 in range(0, NDC, 2):
                        C_queue.append((jd, mc0))

            for i, (j, kc, h) in enumerate(blocks):
                # hard dependency: A(j) filler fully emitted before B(j)
                while fpos <= a_done_idx[j]:
                    have_filler_pe += filler[fpos][-1]
                    run_filler(filler[fpos])
                    fpos += 1
                # pacing: A-filler front-loaded (exhausts ~60% through the
                # block stream); C spread over the remainder
                frac = bw_done / total_bw
                want_filler = total_filler_pe * min(1.0, frac / 0.60)
                while have_filler_pe < want_filler and fpos < len(filler):
                    have_filler_pe += filler[fpos][-1]
                    run_filler(filler[fpos])
                    fpos += 1
                want_c = int(total_c_pairs * max(0.0, (frac - 0.18) / 0.77))
                while c_emitted < want_c and C_queue:
                    jc, mc0 = C_queue.pop(0)
                    emit_C(jc, mc0)
                    c_emitted += 1
                if i >= LOOK:
                    jj_, kc_, h_ = blocks[i - LOOK]
                    emit_rest(jj_, kc_, h_)
                    after_rest(jj_)
                emit_scores(j, kc, h)
                bw_done += bw_of(j, kc)
            while fpos < len(filler):
                run_filler(filler[fpos])
                fpos += 1
            for (j, kc, h) in blocks[len(blocks) - LOOK:]:
                emit_rest(j, kc, h)
                after_rest(j)
            while C_queue:
                jc, mc0 = C_queue.pop(0)
                emit_C(jc, mc0)
    nc.compile()
    return nc


def _host_constants():
    inv = 1.0 / (10000.0 ** (np.arange(0, DH, 2, dtype=np.float32) / DH))
    t = np.arange(S, dtype=np.float32)
    fr = t[:, None] * inv[None, :]                       # [S, DH/2]
    cosT = np.repeat(np.cos(fr).astype(np.float32).T, 2, axis=0)  # [DH, S]
    sinT = np.repeat(np.sin(fr).astype(np.float32).T, 2, axis=0)

    # tri[k, c] = 0 if k <= c else MASK_NEG  (in-diagonal-block causal mask)
    kk = np.arange(128)[:, None]
    cc = np.arange(128)[None, :]
    tri = np.where(kk <= cc, 0.0, MASK_NEG).astype(np.float32)

    # pt = P.T with P@x the rotate-half shuffle
    pt = np.zeros((128, 128), dtype=np.float32)
    i = np.arange(0, 128, 2)
    pt[i + 1, i] = -1.0
    pt[i, i + 1] = 1.0

    ident = np.eye(128, dtype=np.float32)
    onesk = np.ones((128, 128), dtype=np.float32)
    return cosT, sinT, tri, pt, ident, onesk


def _run(inputs, trace=False, trace_kwargs=None):
    from concourse.bass_utils import run_bass_kernel_spmd

    if "nc" not in _CACHE:
        _CACHE["nc"] = _build_program()
    nc = _CACHE["nc"]

    resid_pre = np.asarray(inputs["resid_pre"], dtype=np.float32)
    W_Q = np.asarray(inputs["W_Q"], dtype=np.float32)
    W_K = np.asarray(inputs["W_K"], dtype=np.float32)
    W_V = np.asarray(inputs["W_V"], dtype=np.float32)
    W_O = np.asarray(inputs["W_O"], dtype=np.float32)
    b_Q = np.asarray(inputs["b_Q"], dtype=np.float32)
    b_K = np.asarray(inputs["b_K"], dtype=np.float32)
    b_V = np.asarray(inputs["b_V"], dtype=np.float32)
    b_O = np.asarray(inputs["b_O"], dtype=np.float32)

    xt = np.ascontiguousarray(resid_pre[0].T)
    cosT, sinT, tri, pt, ident, onesk = _host_constants()

    in_maps = []
    for c in range(NC):
        hs = slice(c * HPC, (c + 1) * HPC)
        # wall stream slots per dc: [q0 k0 v0 v1 q1 k1] (consumption order)
        wl = np.empty((128, NDC, 6, DH), dtype=np.float32)
        slot = ((0, 1, 2), (4, 5, 3))
        for h in range(HPC):
            for ki, W in enumerate((W_Q, W_K, W_V)):
                wh = W[c * HPC + h].reshape(NDC, 128, DH)  # [dc, p, e]
                wl[:, :, slot[h][ki], :] = wh.transpose(1, 0, 2)
        in_maps.append({
            "xt": xt,
            "wall": np.ascontiguousarray(wl.reshape(128, NDC * 6 * DH)),
            "wo": np.ascontiguousarray(W_O[hs]),
            "bq": np.ascontiguousarray(b_Q[hs][:, :, None]),
            "bk": np.ascontiguousarray(b_K[hs][:, :, None]),
            "bv": np.ascontiguousarray(b_V[hs][:, :, None]),
            "cost": cosT, "sint": sinT, "tri": tri, "pt": pt,
            "ident": ident, "onesk": onesk,
        })

    kw = dict(trace_kwargs or {})
    last_err = None
    for attempt in range(3):
        try:
            res = run_bass_kernel_spmd(nc, in_maps, list(range(NC)), trace=trace, **kw)
            break
        except Exception as e:  # transient NRT wedges clear on retry
            last_err = e
            if attempt == 2 or "UNRECOVERABLE" not in str(e).upper() and "UNAVAILABLE" not in str(e).upper():
                raise
            import time
            time.sleep(3.0)
    else:
        raise last_err

    acc = np.zeros((D, S), dtype=np.float32)
    for c in range(NC):
        acc += np.asarray(res.results[c]["outt"], dtype=np.float32)
    out = acc.T + b_O[None, :]
    return out.reshape(1, S, D).astype(np.float32), res


def kernel(**inputs) -> np.ndarray:
    out, _ = _run(inputs, trace=False)
    return out


# revision 53
# speedup vs baseline: 1.1146x; 1.1146x over previous
"""Trainium2 Bass kernel for nn_Attention_10015863734775.

Multi-head causal attention (16 heads, d_model 2048, d_head 128, seq 2048,
batch 1) with llama-style interleaved RoPE and a signed-softmax:
    attn_w = sign(s) * exp(|s| - max|s|);  attn = attn_w / (sum|attn_w| + 1e-6)
The max-subtraction cancels in the normalization, so the device computes
attn = sign(s)exp(|s|) / sum exp(|s|).

Sharding: 2 heads per NeuronCore (8 cores). Each core receives the full
transposed residual X^T plus its head slices of W_Q/K/V/W_O and computes a
partial output projection outT_c[m, s]; the host sums the 8 partials,
transposes, and adds b_O.

This version keeps the PE saturated end-to-end (the HAM throttle un-gates
only under continuous PE activity): the QKV-projection (A), attention (B)
and output-projection (C) phases are emitted as one interleaved stream.
A-phase runs one head at a time in two passes, q/k then v (2 PSUM
accumulator banks), so B's z/denom
accumulators can coexist; PSUM budget is exactly 8 banks:
  a0 a1 | s s (scores/rope-shuffle/v-transpose/psO ring) | z0 z1 | d0 d1
Each head's denominator accumulates broadcast across a full bank (ones
stationary), so its fast-approx reciprocal is directly the broadcast
normalizer.
Elementwise work is greedily balanced across ACT / DVE / GpSimd.
"""

import math

import numpy as np

S = 2048          # sequence length
D = 2048          # d_model
DH = 128          # d_head
NH = 16           # total heads
NC = 8            # neuron cores
HPC = NH // NC    # heads per core (2)
ST = 512          # seq tile (matmul free dim / one PSUM bank)
NST = S // ST     # 4 seq tiles
NDC = D // 128    # 16 contraction chunks
C_SCALE = 1.0 / math.sqrt(float(DH))
MASK_NEG = -1.0e5

_CACHE = {}


def _build_program():
    import concourse.tile as tile
    from concourse import bacc, mybir

    F32 = mybir.dt.float32
    F32R = mybir.dt.float32r
    U32 = mybir.dt.uint32
    AF = mybir.ActivationFunctionType
    ALU = mybir.AluOpType

    nc = bacc.Bacc("TRN2", target_bir_lowering=False, debug=False, num_devices=NC)

    xt_d = nc.dram_tensor("xt", [D, S], F32, kind="ExternalInput").ap()
    wall_d = nc.dram_tensor("wall", [128, NDC * 6 * DH], F32, kind="ExternalInput").ap()
    wo_d = nc.dram_tensor("wo", [HPC, DH, D], F32, kind="ExternalInput").ap()
    bq_d = nc.dram_tensor("bq", [HPC, DH, 1], F32, kind="ExternalInput").ap()
    bk_d = nc.dram_tensor("bk", [HPC, DH, 1], F32, kind="ExternalInput").ap()
    bv_d = nc.dram_tensor("bv", [HPC, DH, 1], F32, kind="ExternalInput").ap()
    cos_d = nc.dram_tensor("cost", [DH, S], F32, kind="ExternalInput").ap()
    sin_d = nc.dram_tensor("sint", [DH, S], F32, kind="ExternalInput").ap()
    tri_d = nc.dram_tensor("tri", [128, 128], F32, kind="ExternalInput").ap()
    pt_d = nc.dram_tensor("pt", [128, 128], F32, kind="ExternalInput").ap()
    id_d = nc.dram_tensor("ident", [128, 128], F32, kind="ExternalInput").ap()
    ok_d = nc.dram_tensor("onesk", [128, 128], F32, kind="ExternalInput").ap()
    BF16 = mybir.dt.bfloat16
    out_d = nc.dram_tensor("outt", [D, S], BF16, kind="ExternalOutput").ap()

    # engine load model (us per full-width [128,512] pass) for greedy balance
    load = {"act": 0.0, "dve": 0.0, "gp": 0.0}

    def pick(*opts):
        e, c = min(opts, key=lambda ec: load[ec[0]] + ec[1])
        load[e] += c
        return e

    with tile.TileContext(nc) as tc:
        with tc.tile_pool(name="persist", bufs=1) as pp, \
             tc.tile_pool(name="sp", bufs=1) as sp, \
             tc.tile_pool(name="psA", bufs=1, space="PSUM") as psA, \
             tc.tile_pool(name="psS", bufs=2, space="PSUM") as psS, \
             tc.tile_pool(name="psZ", bufs=1, space="PSUM") as psZ, \
             tc.tile_pool(name="psD", bufs=1, space="PSUM") as psD:

            # ---------------- persistent SBUF -----------------------------
            wall_sb = pp.tile([128, NDC * 6 * DH], F32R, tag="wall", name="wall_sb")
            krot = [pp.tile([DH, S], F32R, tag=f"krot{h}", name=f"krot{h}")
                    for h in range(HPC)]
            v_sb = [pp.tile([128, NDC * DH], F32R, tag=f"v{h}", name=f"v{h}")
                    for h in range(HPC)]
            wo_sb = [pp.tile([DH, D], F32R, tag=f"wo{h}", name=f"wo{h}")
                     for h in range(HPC)]
            bqs, bks, bvs = [], [], []
            for h in range(HPC):
                for lst, dd, nm in ((bqs, bq_d, "bq"), (bks, bk_d, "bk"),
                                    (bvs, bv_d, "bv")):
                    bt = pp.tile([DH, 1], F32, tag=f"{nm}{h}", name=f"{nm}{h}")
                    lst.append(bt)
            tri_sb = pp.tile([128, 128], F32, tag="tri", name="tri_sb")
            pt_sb = pp.tile([128, 128], F32R, tag="pt", name="pt_sb")
            id_sb = pp.tile([128, 128], F32R, tag="ident", name="id_sb")
            ok_sb = pp.tile([128, 128], F32R, tag="onesk", name="ok_sb")
            cabs_sb = pp.tile([128, 1], U32, tag="cabs", name="cabs_sb")
            nc.vector.memset(cabs_sb[:], 0x7FFFFFFF)
            csgn_sb = pp.tile([128, 1], U32, tag="csgn", name="csgn_sb")
            nc.vector.memset(csgn_sb[:], 0x80000000)
            cone_sb = pp.tile([128, 1], U32, tag="cone", name="cone_sb")
            nc.vector.memset(cone_sb[:], 0x3F800000)
            zero_sb = pp.tile([128, 1], F32, tag="zero", name="zero_sb")
            nc.vector.memset(zero_sb[:], 0.0)
            import os as _os
            _nonce = float(int(_os.environ.get("KBUILD_NONCE", "0")))
            nonce_sb = pp.tile([128, 1], F32, tag="nonce", name="nonce_sb")
            nc.vector.memset(nonce_sb[:], _nonce)

            CH = 768  # f32 cols per dc chunk (6*128)

            def dma_wall(dc):
                # one 768-col chunk per dc on the scalar hwdge queue (smaller
                # pieces pay ~700ns fixed overhead each and clog the queue)
                o = dc * CH
                nc.scalar.dma_start(wall_sb[:, o:o + CH],
                                    wall_d[:, o:o + CH].bitcast(F32R))

            _wix = ((0, 1, 2), (4, 5, 3))  # [h][q,k,v] -> stream slot

            def wsl(dc, h, key):
                o = (dc * 6 + _wix[h][key]) * 128
                return wall_sb[:, o:o + 128]

            # ---------------- working pools --------------------------------
            # xt tiles for seq-tile st stay live across both head sub-bursts
            xt_p = sp  # tags below carry bufs explicitly
            qrot_p = sp
            trig_p = sp
            work_p = sp
            out_p = sp

            xt_tiles = {}     # (st, dc) -> tile (ring bufs=17)
            qrot_t = {}       # (st, h) -> tile
            znt_t = {}        # (j, h) -> tile
            cos_t = {}        # st -> tile
            sin_t = {}

            def dma_xt(st, dc):
                t = xt_p.tile([128, ST], F32R, tag="xt", bufs=17,
                              name=f"xt{st}_{dc}")
                nc.sync.dma_start(
                    t[:], xt_d[dc * 128:(dc + 1) * 128,
                               st * ST:(st + 1) * ST].bitcast(F32R))
                xt_tiles[(st, dc)] = t

            def dma_trig(st):
                ssl = slice(st * ST, (st + 1) * ST)
                ct = trig_p.tile([DH, ST], F32, tag="cos", bufs=2, name=f"cos{st}")
                nc.scalar.dma_start(ct[:], cos_d[:, ssl])
                st_t = trig_p.tile([DH, ST], F32, tag="sin", bufs=2, name=f"sin{st}")
                nc.scalar.dma_start(st_t[:], sin_d[:, ssl])
                cos_t[st] = ct
                sin_t[st] = st_t

            # ---------------- A phase work units ---------------------------
            acc = {}

            def a_mm_qk(st, h, dc):
                if dc == 0:
                    acc["q"] = psA.tile([128, ST], F32, tag="a0", name=f"aq{st}_{h}")
                    acc["k"] = psA.tile([128, ST], F32, tag="a1", name=f"ak{st}_{h}")
                xt_t = xt_tiles[(st, dc)]
                for ki, key in enumerate(("q", "k")):
                    nc.tensor.matmul(acc[key][:], wsl(dc, h, ki), xt_t[:],
                                     start=(dc == 0), stop=(dc == NDC - 1))

            def a_mm_v(st, h, dc):
                if dc == 0:
                    acc["v"] = psA.tile([128, ST], F32, tag="a0", name=f"av{st}_{h}")
                xt_t = xt_tiles[(st, dc)]
                nc.tensor.matmul(acc["v"][:], wsl(dc, h, 2), xt_t[:],
                                 start=(dc == 0), stop=(dc == NDC - 1))

            def a_tail_qk(st, h):
                """q/k evictions + rope for (st, h)"""
                ssl = slice(st * ST, (st + 1) * ST)
                for key, bias in (("q", bqs[h]), ("k", bks[h])):
                    x_sb = work_p.tile([128, ST], F32R, tag="ev", bufs=3,
                                       name=f"ev{key}{h}_{st}")
                    if st == 0 or pick(("dve", 0.78), ("act", 0.70)) == "dve":
                        if st == 0:
                            load["dve"] += 0.78
                        nc.vector.tensor_scalar(x_sb[:], acc[key][:], bias[:],
                                                None, ALU.add)
                    else:
                        nc.scalar.activation(x_sb[:], acc[key][:], AF.Identity,
                                             bias=bias[:])
                    if key == "q":
                        dst = qrot_p.tile([DH, ST], F32R, tag=f"qr{h}", bufs=2,
                                          name=f"qr{h}_{st}")
                        qrot_t[(st, h)] = dst
                        dsl = slice(0, ST)
                    else:
                        dst = krot[h]
                        dsl = ssl
                    shuf = psS.tile([128, ST], F32, tag="s", name=f"sh{key}{h}_{st}")
                    nc.tensor.matmul(shuf[:], pt_sb[:], x_sb[:], start=True,
                                     stop=True)
                    t1 = work_p.tile([128, ST], F32, tag="t1", bufs=2,
                                     name=f"t1{key}{h}_{st}")
                    load["gp"] += 1.45
                    nc.gpsimd.tensor_tensor(t1[:], x_sb[:].bitcast(F32),
                                            cos_t[st][:], ALU.mult)
                    t2 = work_p.tile([128, ST], F32, tag="t2", bufs=2,
                                     name=f"t2{key}{h}_{st}")
                    load["dve"] += 0.90
                    nc.vector.tensor_tensor(t2[:], shuf[:], sin_t[st][:], ALU.mult)
                    if pick(("dve", 0.90), ("gp", 1.45)) == "dve":
                        nc.vector.tensor_tensor(dst[:, dsl], t1[:], t2[:], ALU.add)
                    else:
                        nc.gpsimd.tensor_tensor(dst[:, dsl], t1[:], t2[:], ALU.add)

            def a_tail_v(st, h):
                """v eviction + transpose for (st, h)"""
                vt_sb = work_p.tile([128, ST], F32R, tag="evv", bufs=2,
                                    name=f"evv{h}_{st}")
                load["dve"] += 0.78
                nc.vector.tensor_scalar(vt_sb[:], acc["v"][:], bvs[h][:], None,
                                        ALU.add)
                vtr = psS.tile([128, ST], F32, tag="s", name=f"vtr{h}_{st}")
                for sc in range(4):
                    nc.tensor.transpose(
                        vtr[:, sc * 128:(sc + 1) * 128].bitcast(F32R),
                        vt_sb[:, sc * 128:(sc + 1) * 128], id_sb[:])
                if pick(("dve", 0.78), ("act", 0.72)) == "dve":
                    nc.vector.tensor_copy(
                        v_sb[h][:, st * ST:(st + 1) * ST], vtr[:])
                else:
                    nc.scalar.activation(
                        v_sb[h][:, st * ST:(st + 1) * ST], vtr[:], AF.Copy)

            # ---------------- B phase work units ---------------------------
            pss_map = {}
            psz = {}
            psd = {}

            def emit_scores(j, kc, h):
                jj = kc - 4 * j
                lo = jj * 128 if 0 <= jj < 4 else 0
                pss = psS.tile([128, ST], F32, tag="s", name=f"s{h}_{j}_{kc}")
                nc.tensor.matmul(pss[:, lo:], krot[h][:, kc * 128:(kc + 1) * 128],
                                 qrot_t[(j, h)][:, lo:], start=True, stop=True)
                pss_map[(j, kc, h)] = (pss, lo)

            def emit_rest(j, kc, h):
                pss, lo = pss_map.pop((j, kc, h))
                if kc == 0:
                    psz[(j, h)] = psZ.tile([128, ST], F32, tag=f"z{h}",
                                           name=f"z{h}_{j}")
                    psd[(j, h)] = psD.tile([128, ST], F32, tag=f"d{h}",
                                           name=f"d{h}_{j}")
                lsl = slice(lo, ST)
                wdt = ST - lo
                f = wdt / ST
                a = work_p.tile([128, ST], F32, tag="a", bufs=2,
                                name=f"a{h}_{j}_{kc}")
                if pick(("dve", 0.78 * f), ("act", 0.70 * f)) == "dve":
                    nc.vector.tensor_scalar(a[:, lsl].bitcast(U32),
                                            pss[:, lsl].bitcast(U32),
                                            cabs_sb[:], None, ALU.bitwise_and)
                    exp_scale = C_SCALE
                else:
                    nc.scalar.activation(a[:, lsl], pss[:, lsl], AF.Abs,
                                         bias=zero_sb[:], scale=C_SCALE)
                    exp_scale = 1.0
                jj = kc - 4 * j
                if 0 <= jj < 4:
                    load["gp"] += 0.42
                    nc.gpsimd.tensor_tensor(a[:, lo:lo + 128], a[:, lo:lo + 128],
                                            tri_sb[:], ALU.add)
                # sign: on the engine opposite the abs
                sg = work_p.tile([128, ST], F32, tag="sg", bufs=2,
                                 name=f"sg{h}_{j}_{kc}")
                if pick(("dve", 0.78 * f), ("act", 0.70 * f)) == "dve":
                    nc.vector.tensor_scalar(sg[:, lsl].bitcast(U32),
                                            pss[:, lsl].bitcast(U32),
                                            csgn_sb[:], cone_sb[:],
                                            ALU.bitwise_and, ALU.bitwise_or)
                else:
                    nc.scalar.activation(sg[:, lsl], pss[:, lsl], AF.Sign,
                                         bias=zero_sb[:])
                e2 = work_p.tile([128, ST], F32R, tag="e2", bufs=4,
                                 name=f"e2{h}_{j}_{kc}")
                load["act"] += 0.70 * f
                nc.scalar.activation(e2[:, lsl], a[:, lsl], AF.Exp,
                                     bias=zero_sb[:], scale=exp_scale)
                w = work_p.tile([128, ST], F32R, tag="w", bufs=4,
                                name=f"w{h}_{j}_{kc}")
                if pick(("dve", 0.90 * f), ("gp", 1.45 * f)) == "dve":
                    nc.vector.tensor_tensor(w[:, lsl], sg[:, lsl],
                                            e2[:, lsl].bitcast(F32), ALU.mult)
                else:
                    nc.gpsimd.tensor_tensor(w[:, lsl], sg[:, lsl],
                                            e2[:, lsl].bitcast(F32), ALU.mult)
                nkc_ = 4 * (j + 1)
                nc.tensor.matmul(psd[(j, h)][:, lsl], ok_sb[:],
                                 e2[:, lsl], start=(kc == 0), stop=(kc == nkc_ - 1))
                nc.tensor.matmul(psz[(j, h)][:, lsl],
                                 v_sb[h][:, kc * 128:(kc + 1) * 128], w[:, lsl],
                                 start=(kc == 0), stop=(kc == nkc_ - 1))

            def finalize(j, h):
                rb_sb = work_p.tile([128, ST], F32, tag="rb", bufs=1,
                                    name=f"rbs{h}_{j}")
                load["dve"] += 0.62
                nc.vector.reciprocal_approx_fast(out=rb_sb[:],
                                                 in_=psd.pop((j, h))[:])
                zt = qrot_p.tile([DH, ST], F32R, tag=f"znt{h}", bufs=2,
                                 name=f"znt{h}_{j}")
                znt_t[(j, h)] = zt
                load["dve"] += 0.90
                nc.vector.tensor_tensor(zt[:], psz.pop((j, h))[:], rb_sb[:],
                                        ALU.mult)

            def emit_C(j, mc0):
                jsl = slice(j * ST, (j + 1) * ST)
                for mc in (mc0, mc0 + 1):
                    pso = psS.tile([128, ST], F32, tag="s", name=f"o{j}_{mc}")
                    for h in range(HPC):
                        nc.tensor.matmul(pso[:],
                                         wo_sb[h][:, mc * 128:(mc + 1) * 128],
                                         znt_t[(j, h)][:], start=(h == 0),
                                         stop=(h == HPC - 1))
                    o_sb = out_p.tile([128, ST], BF16, tag="o", bufs=2,
                                      name=f"ev{j}_{mc}")
                    if pick(("dve", 0.55), ("act", 0.55)) == "dve":
                        nc.vector.tensor_copy(o_sb[:], pso[:])
                    else:
                        nc.scalar.activation(o_sb[:], pso[:], AF.Copy)
                    nc.sync.dma_start(out_d[mc * 128:(mc + 1) * 128, jsl], o_sb[:])

            # ---------------- global schedule ------------------------------
            # PE warm-up: the HAM throttle needs ~3.4us of continuous PE
            # activity to lift to full rate, and the first real matmul can't
            # start until wall(0)+xt(0,0) land (~9us). Spin the PE on a dummy
            # tile so the p-state/HAM are warm when real work arrives.
            dumf = work_p.tile([128, ST], F32, tag="dumf", bufs=1, name="dumf")
            nc.vector.memset(dumf[:], 1.0)
            dum = work_p.tile([128, ST], F32R, tag="dum", bufs=1, name="dum")
            nc.vector.tensor_copy(dum[:], dumf[:])
            for i in range(30):
                pswu = psS.tile([128, ST], F32, tag="s", name=f"wu{i}")
                nc.tensor.matmul(pswu[:, :128], dum[:, :128], dum[:, :128],
                                 start=True, stop=True)

            # prologue DMAs. scalar hwdge: wall chunks paced to the A(0)
            # consumption, with the early-needed consts slotted between.
            # sync hwdge: xt st=0 then biases/tri/ok. swdge: wo only.
            for dc in range(NDC):
                dma_xt(0, dc)
            for lst, dd in ((bqs, bq_d), (bks, bk_d), (bvs, bv_d)):
                for h in range(HPC):
                    nc.sync.dma_start(lst[h][:], dd[h])
            nc.sync.dma_start(tri_sb[:], tri_d[:])
            nc.sync.dma_start(ok_sb[:], ok_d[:].bitcast(F32R))
            for dc in range(3):
                dma_wall(dc)
            nc.scalar.dma_start(pt_sb[:], pt_d[:].bitcast(F32R))
            nc.scalar.dma_start(id_sb[:], id_d[:].bitcast(F32R))
            for dc in range(3, 6):
                dma_wall(dc)
            dma_trig(0)
            for dc in range(6, NDC):
                dma_wall(dc)
            for h in range(HPC):
                nc.gpsimd.dma_start(wo_sb[h][:], wo_d[h].bitcast(F32R))

            # A(0, h0): wall-DMA-paced; fill the chase gaps with dummy MMs so
            # the PE/HAM stay warm while weight chunks stream in.
            for dc in range(NDC):
                a_mm_qk(0, 0, dc)
                if dc < 15:
                    for r in range(2):
                        pswu = psS.tile([128, ST], F32, tag="s", name=f"wf{dc}_{r}")
                        nc.tensor.matmul(pswu[:], dum[:, :128], dum[:],
                                         start=True, stop=True)
            a_tail_qk(0, 0)
            for dc in range(NDC):
                a_mm_v(0, 0, dc)
            a_tail_v(0, 0)

            # ---- globally-paced stream: B blocks paced against one filler
            # stream (A(0,h1), A(1..3), DMAs) plus a C queue gated on fin(j).
            # B(j) blocks require A(<=j) fully emitted (qrot/krot/v deps).
            filler = []
            # A(0, h1)
            for dc in range(NDC):
                filler.append(("aqk", 0, 1, dc, 0.43))
            filler.append(("atailqk", 0, 1, 0.45))
            for dc in range(NDC):
                filler.append(("av", 0, 1, dc, 0.21))
            filler.append(("atailv", 0, 1, 0.35))
            filler.append(("trig", 1, 0.0))
            # A(1..3)
            for stn in range(1, NST):
                for h in range(HPC):
                    for dc in range(NDC):
                        if h == 0:
                            filler.append(("xt", stn, dc, 0.0))
                        filler.append(("aqk", stn, h, dc, 0.43))
                    filler.append(("atailqk", stn, h, 0.45))
                    for dc in range(NDC):
                        filler.append(("av", stn, h, dc, 0.21))
                    filler.append(("atailv", stn, h, 0.35))
                if stn + 1 < NST:
                    filler.append(("trig", stn + 1, 0.0))
            # index of the last filler unit belonging to A(j) (for dep gating)
            a_done_idx = {}
            for i, u in enumerate(filler):
                if u[0] == "atailv":
                    a_done_idx[u[1]] = i
            total_filler_pe = sum(u[-1] for u in filler)

            def run_filler(u):
                kind = u[0]
                if kind == "xt":
                    dma_xt(u[1], u[2])
                elif kind == "aqk":
                    a_mm_qk(u[1], u[2], u[3])
                elif kind == "av":
                    a_mm_v(u[1], u[2], u[3])
                elif kind == "atailqk":
                    a_tail_qk(u[1], u[2])
                elif kind == "atailv":
                    a_tail_v(u[1], u[2])
                elif kind == "trig":
                    dma_trig(u[1])

            blocks = [(j, kc, h) for j in range(NST)
                      for kc in range(4 * (j + 1)) for h in range(HPC)]
            # block weight = elementwise width fraction
            def bw_of(j, kc):
                jj = kc - 4 * j
                return 1.0 - jj * 0.25 if 0 <= jj < 4 else 1.0
            total_bw = sum(bw_of(j, kc) for j, kc, h in blocks)

            C_queue = []     # (j, mc0) ready to emit
            c_emitted = 0
            total_c_pairs = NST * (NDC // 2)
            done_in_j = {j: 0 for j in range(NST)}
            LOOK = 3
            fpos = 0
            have_filler_pe = 0.0
            bw_done = 0.0

            def after_rest(jd):
                done_in_j[jd] += 1
                if done_in_j[jd] == 2 * 4 * (jd + 1):
                    for h in range(HPC):
                        finalize(jd, h)
                    for mc0 in range(0, NDC, 2):
                        C_queue.append((jd, mc0))

            for i, (j, kc, h) in enumerate(blocks):
                # hard dependency: A(j) filler fully emitted before B(j)
                while fpos <= a_done_idx[j]:
                    have_filler_pe += filler[fpos][-1]
                    run_filler(filler[fpos])
                    fpos += 1
                # pacing: A-filler front-loaded (exhausts ~60% through the
                # block stream); C spread over the remainder
                frac = bw_done / total_bw
                want_filler = total_filler_pe * min(1.0, frac / 0.60)
                while have_filler_pe < want_filler and fpos < len(filler):
                    have_filler_pe += filler[fpos][-1]
                    run_filler(filler[fpos])
                    fpos += 1
                want_c = int(total_c_pairs * max(0.0, (frac - 0.18) / 0.77))
                while c_emitted < want_c and C_queue:
                    jc, mc0 = C_queue.pop(0)
                    emit_C(jc, mc0)
                    c_emitted += 1
                if i >= LOOK:
                    jj_, kc_, h_ = blocks[i - LOOK]
                    emit_rest(jj_, kc_, h_)
                    after_rest(jj_)
                emit_scores(j, kc, h)
                bw_done += bw_of(j, kc)
            while fpos < len(filler):
                run_filler(filler[fpos])
                fpos += 1
            for (j, kc, h) in blocks[len(blocks) - LOOK:]:
                emit_rest(j, kc, h)
                after_rest(j)
            while C_queue:
                jc, mc0 = C_queue.pop(0)
                emit_C(jc, mc0)
    nc.compile()
    return nc


def _host_constants():
    inv = 1.0 / (10000.0 ** (np.arange(0, DH, 2, dtype=np.float32) / DH))
    t = np.arange(S, dtype=np.float32)
    fr = t[:, None] * inv[None, :]                       # [S, DH/2]
    cosT = np.repeat(np.cos(fr).astype(np.float32).T, 2, axis=0)  # [DH, S]
    sinT = np.repeat(np.sin(fr).astype(np.float32).T, 2, axis=0)

    # tri[k, c] = 0 if k <= c else MASK_NEG  (in-diagonal-block causal mask)
    kk = np.arange(128)[:, None]
    cc = np.arange(128)[None, :]
    tri = np.where(kk <= cc, 0.0, MASK_NEG).astype(np.float32)

    # pt = P.T with P@x the rotate-half shuffle
    pt = np.zeros((128, 128), dtype=np.float32)
    i = np.arange(0, 128, 2)
    pt[i + 1, i] = -1.0
    pt[i, i + 1] = 1.0

    ident = np.eye(128, dtype=np.float32)
    onesk = np.ones((128, 128), dtype=np.float32)
    return cosT, sinT, tri, pt, ident, onesk


def _run(inputs, trace=False, trace_kwargs=None):
    from concourse.bass_utils import run_bass_kernel_spmd

    if "nc" not in _CACHE:
        _CACHE["nc"] = _build_program()
    nc = _CACHE["nc"]

    resid_pre = np.asarray(inputs["resid_pre"], dtype=np.float32)
    W_Q = np.asarray(inputs["W_Q"], dtype=np.float32)
    W_K = np.asarray(inputs["W_K"], dtype=np.float32)
    W_V = np.asarray(inputs["W_V"], dtype=np.float32)
    W_O = np.asarray(inputs["W_O"], dtype=np.float32)
    b_Q = np.asarray(inputs["b_Q"], dtype=np.float32)
    b_K = np.asarray(inputs["b_K"], dtype=np.float32)
    b_V = np.asarray(inputs["b_V"], dtype=np.float32)
    b_O = np.asarray(inputs["b_O"], dtype=np.float32)

    xt = np.ascontiguousarray(resid_pre[0].T)
    cosT, sinT, tri, pt, ident, onesk = _host_constants()

    in_maps = []
    for c in range(NC):
        hs = slice(c * HPC, (c + 1) * HPC)
        # wall stream slots per dc: [q0 k0 v0 v1 q1 k1] (consumption order)
        wl = np.empty((128, NDC, 6, DH), dtype=np.float32)
        slot = ((0, 1, 2), (4, 5, 3))
        for h in range(HPC):
            for ki, W in enumerate((W_Q, W_K, W_V)):
                wh = W[c * HPC + h].reshape(NDC, 128, DH)  # [dc, p, e]
                wl[:, :, slot[h][ki], :] = wh.transpose(1, 0, 2)
        in_maps.append({
            "xt": xt,
            "wall": np.ascontiguousarray(wl.reshape(128, NDC * 6 * DH)),
            "wo": np.ascontiguousarray(W_O[hs]),
            "bq": np.ascontiguousarray(b_Q[hs][:, :, None]),
            "bk": np.ascontiguousarray(b_K[hs][:, :, None]),
            "bv": np.ascontiguousarray(b_V[hs][:, :, None]),
            "cost": cosT, "sint": sinT, "tri": tri, "pt": pt,
            "ident": ident, "onesk": onesk,
        })

    kw = dict(trace_kwargs or {})
    last_err = None
    for attempt in range(3):
        try:
            res = run_bass_kernel_spmd(nc, in_maps, list(range(NC)), trace=trace, **kw)
            break
        except Exception as e:  # transient NRT wedges clear on retry
            last_err = e
            if attempt == 2 or "UNRECOVERABLE" not in str(e).upper() and "UNAVAILABLE" not in str(e).upper():
                raise
            import time
            time.sleep(3.0)
    else:
        raise last_err

    acc = np.zeros((D, S), dtype=np.float32)
    for c in range(NC):
        acc += np.asarray(res.results[c]["outt"], dtype=np.float32)
    out = acc.T + b_O[None, :]
    return out.reshape(1, S, D).astype(np.float32), res


def kernel(**inputs) -> np.ndarray:
    out, _ = _run(inputs, trace=False)
    return out
